# revision 6
# baseline (speedup 1.0000x reference)
"""CoarseMatching (retrieval kNN) kernel for 8x Trainium2 NeuronCores.

Problem: ref[8192,256], src[8192,256] (unit-norm rows, fp32).
  sim = ref @ src.T                      [8192, 8192]
  scores = exp(2*sim - 2)                (monotone in sim)
  outputs: global top-k (k=num_proposal) of scores (row idx, col idx, score)
           + per-row argmax over src.

Strategy:
  - Shard ref rows across 8 cores (1024 rows each); src replicated.
  - Device (per core): bf16 matmul (fp32 PSUM accumulation) of its
    [1024 x 8192] sim block. PSUM drain is split between ScalarE (copies
    half the chunks to SBUF) and VectorE (fused tensor_tensor_reduce of
    the PSUM chunk with the copied partner chunk) producing per-row,
    per-1024-column chunk maxes "cm" [1024 x 8]. Only cm leaves the device.
  - Host: candidate selection from cm with a safety margin that dominates
    the bf16 rounding error, then exact fp64 recomputation of only the
    few hundred candidate chunks (BLAS) for exact top-k / argmax.

  Device cm error vs true fp32 sims is bounded by bf16 input rounding
  (~6e-4 absolute); MARGIN=2e-2 makes candidate selection exact.
"""

import sys

sys.path.insert(0, "/opt/trn_rl_repo")

import numpy as np
import ml_dtypes

N_CORES = 8
N, M, C = 8192, 8192, 256
ROWS_PER_CORE = N // N_CORES          # 1024
STRIPS = ROWS_PER_CORE // 128         # 8 strips of 128 rows
BANK = 512                            # fp32 elems per PSUM bank
N_PAIRS = M // (2 * BANK)             # 8 chunk-pairs of 1024 columns
CM_CHUNK = 2 * BANK                   # cm granularity (1024 columns)
MARGIN = 2e-2                         # >> bf16 matmul error (~6e-4)

_compiled = None


def _build_bass():
    from contextlib import ExitStack
    import concourse.bacc as bacc
    import concourse.tile as tile
    from concourse import mybir

    nc = bacc.Bacc("TRN2", target_bir_lowering=False, debug=False)
    bf16 = mybir.dt.bfloat16
    f32 = mybir.dt.float32
    MAX = mybir.AluOpType.max

    # lhsT k-tiles: [2, 128, 1024] (contract dim on partitions)
    ref_t = nc.declare_dram_parameter("ref_t", [2, 128, ROWS_PER_CORE], bf16, isOutput=False)
    # rhs k-tiles quartered for load/compute overlap: [2, 4, 128, 2048]
    src_t = nc.declare_dram_parameter("src_t", [2, 4, 128, M // 4], bf16, isOutput=False)
    # out: per chunk-pair t, [128 partitions, 8 strips] of 1024-wide chunk maxes
    cm_out = nc.declare_dram_parameter("cm", [N_PAIRS, 128, STRIPS], f32, isOutput=True)

    with tile.TileContext(nc) as tc, ExitStack() as ctx:
        sbuf = ctx.enter_context(tc.tile_pool(name="sbuf", bufs=1))
        sbq_pool = ctx.enter_context(tc.tile_pool(name="sbq", bufs=2))
        scr_pool = ctx.enter_context(tc.tile_pool(name="scr", bufs=2))
        cm_pool = ctx.enter_context(tc.tile_pool(name="cmp", bufs=2))
        psum_p = ctx.enter_context(tc.tile_pool(name="psp", bufs=3, space="PSUM"))
        psum_q = ctx.enter_context(tc.tile_pool(name="psq", bufs=1, space="PSUM"))

        # resident weights (ref^T) per k-tile, then src^T quarters
        reft = [sbuf.tile([128, ROWS_PER_CORE], bf16, name=f"reft{k}") for k in range(2)]
        for k in range(2):
            nc.sync.dma_start(reft[k][:], ref_t[k])
        srcq = [
            [sbuf.tile([128, M // 4], bf16, name=f"srcq{k}_{q}") for q in range(4)]
            for k in range(2)
        ]
        for q in range(4):
            for k in range(2):
                nc.sync.dma_start(srcq[k][q][:], src_t[k, q])

        for t in range(N_PAIRS):              # column chunk-pairs (1024 cols)
            j0, j1 = 2 * t, 2 * t + 1
            q0, off0 = j0 // 4, (j0 % 4) * BANK
            q1, off1 = j1 // 4, (j1 % 4) * BANK
            cm_sb = cm_pool.tile([128, STRIPS], f32, name="cm_sb", tag="cm_sb")
            for u in range(STRIPS // 2):      # strip pairs
                ps_p = psum_p.tile([128, 2, BANK], f32, name="ps_p", tag="ps_p")
                ps_q = psum_q.tile([128, 2, BANK], f32, name="ps_q", tag="ps_q")
                for s in range(2):
                    strip = 2 * u + s
                    rsl = slice(strip * 128, (strip + 1) * 128)
                    for k in range(2):
                        nc.tensor.matmul(
                            ps_p[:, s], reft[k][:, rsl],
                            srcq[k][q0][:, off0:off0 + BANK],
                            start=(k == 0), stop=(k == 1),
                        )
                for s in range(2):
                    strip = 2 * u + s
                    rsl = slice(strip * 128, (strip + 1) * 128)
                    for k in range(2):
                        nc.tensor.matmul(
                            ps_q[:, s], reft[k][:, rsl],
                            srcq[k][q1][:, off1:off1 + BANK],
                            start=(k == 0), stop=(k == 1),
                        )
                # ScalarE drains Q to SBUF; VectorE runs a fused running-max
                # scan over the PSUM chunk and its SBUF partner; the scan's
                # last column is the max over both 512-wide chunks.
                sbq = sbq_pool.tile([128, 2, BANK], f32, name="sbq", tag="sbq")
                nc.scalar.copy(sbq[:], ps_q[:])
                scr = scr_pool.tile([128, 2, BANK], f32, name="scr", tag="scr")
                for s in range(2):
                    nc.vector.tensor_tensor_scan(
                        out=scr[:, s],
                        data0=ps_p[:, s],
                        data1=sbq[:, s],
                        initial=-3.0e38,
                        op0=MAX,
                        op1=MAX,
                    )
                nc.vector.tensor_copy(
                    cm_sb[:, 2 * u:2 * u + 2], scr[:, :, BANK - 1:BANK]
                )
            nc.gpsimd.dma_start(cm_out[t], cm_sb[:])

    nc.compile()
    return nc


def _get_compiled():
    global _compiled
    if _compiled is None:
        _compiled = _build_bass()
    return _compiled


def _ensure_ntff_hook():
    """Register the axon NTFF profiling hook if the image's antenv lacks it."""
    try:
        from antenv.axon_hooks import get_axon_ntff_profile_hook  # noqa: F401
        return
    except ImportError:
        pass
    try:
        import types

        sys.path.insert(0, "/root/.axon_site")
        from trn_agent_boot.trn_boot import _ntff_profile_via_ctypes

        hook = _ntff_profile_via_ctypes("/opt/axon/libaxon_pjrt.so")
        m = types.ModuleType("antenv.axon_hooks")
        m._hook = hook
        m.get_axon_ntff_profile_hook = lambda: m._hook
        m.set_axon_ntff_profile_hook = lambda h: setattr(m, "_hook", h)
        sys.modules["antenv.axon_hooks"] = m
        import antenv

        antenv.axon_hooks = m
    except Exception as e:  # profiling is optional; never break the run
        print(f"NTFF hook registration failed: {e}", file=sys.stderr)


def _run_device(ref_f32: np.ndarray, src_f32: np.ndarray, trace: bool = False):
    """Run the SPMD bass kernel; returns cm [N, N_PAIRS] fp32 and the raw results obj."""
    from concourse.bass_utils import run_bass_kernel_spmd

    if trace:
        _ensure_ntff_hook()

    nc = _get_compiled()

    ref_bf = ref_f32.astype(ml_dtypes.bfloat16)
    src_bf = src_f32.astype(ml_dtypes.bfloat16)

    # [C, M] transposed layouts, k-tiled on partitions
    src_tt = np.ascontiguousarray(src_bf.T).reshape(2, 128, M)
    src_tt = np.ascontiguousarray(src_tt.reshape(2, 128, 4, M // 4).transpose(0, 2, 1, 3))

    in_maps = []
    for c in range(N_CORES):
        rows = slice(c * ROWS_PER_CORE, (c + 1) * ROWS_PER_CORE)
        reft = np.ascontiguousarray(ref_bf[rows].T).reshape(2, 128, ROWS_PER_CORE)
        in_maps.append({"ref_t": reft, "src_t": src_tt})

    res = run_bass_kernel_spmd(nc, in_maps, core_ids=list(range(N_CORES)), trace=trace)

    # cm[t, p, i] -> local row = i*128 + p
    cm = np.empty((N, N_PAIRS), dtype=np.float32)
    for c in range(N_CORES):
        block = res.results[c]["cm"]            # [8, 128, 8]
        cm[c * ROWS_PER_CORE:(c + 1) * ROWS_PER_CORE] = (
            block.transpose(2, 1, 0).reshape(ROWS_PER_CORE, N_PAIRS)
        )
    return cm, res


def _recompute_chunks(ref64, src64, pairs):
    """Exact fp64 sims for a set of (row, chunk) pairs.

    Returns dict chunk -> (rows_array, values [len(rows), CM_CHUNK])."""
    out = {}
    pairs = np.asarray(pairs)
    if pairs.size == 0:
        return out
    for j in np.unique(pairs[:, 1]):
        rows = pairs[pairs[:, 1] == j, 0]
        vals = ref64[rows] @ src64[j * CM_CHUNK:(j + 1) * CM_CHUNK].T
        out[int(j)] = (rows, vals)
    return out


def kernel(ref_feats, src_feats, num_proposal):
    ref = np.asarray(ref_feats, dtype=np.float32)
    src = np.asarray(src_feats, dtype=np.float32)
    k = int(num_proposal)

    cm, _ = _run_device(ref, src)

    ref64 = ref.astype(np.float64)
    src64 = src.astype(np.float64)

    # ---- per-row argmax over src (all_ref_corr_indices) ----
    row_best = cm.max(axis=1)
    cand_mask = cm >= (row_best[:, None] - MARGIN)
    rows_r, chunks_r = np.nonzero(cand_mask)
    rec = _recompute_chunks(ref64, src64, np.stack([rows_r, chunks_r], axis=1))
    best_val = np.full(N, -np.inf)
    best_idx = np.zeros(N, dtype=np.int64)
    for j, (rows, vals) in sorted(rec.items()):
        am = vals.argmax(axis=1)
        v = vals[np.arange(len(rows)), am]
        idx = j * CM_CHUNK + am
        upd = v > best_val[rows]
        # strict > keeps the lowest column index on exact ties because
        # chunks are visited in ascending order and argmax takes the first max
        best_val[rows] = np.where(upd, v, best_val[rows])
        best_idx[rows] = np.where(upd, idx, best_idx[rows])
    all_ref_corr_indices = best_idx.astype(np.int32)

    # ---- global top-k ----
    flat_cm = cm.ravel()
    kth = min(k, flat_cm.size)
    t = np.partition(flat_cm, flat_cm.size - kth)[flat_cm.size - kth]
    rows_g, chunks_g = np.nonzero(cm >= t - MARGIN)
    rec = _recompute_chunks(ref64, src64, np.stack([rows_g, chunks_g], axis=1))
    cand_vals = []
    cand_flat = []
    for j, (rows, vals) in sorted(rec.items()):
        cols = j * CM_CHUNK + np.arange(CM_CHUNK)
        cand_vals.append(vals.ravel())
        cand_flat.append((rows[:, None] * M + cols[None, :]).ravel())
    cand_vals = np.concatenate(cand_vals)
    cand_flat = np.concatenate(cand_flat)

    # top-k by value desc, ties -> lower flat index (matches jax.lax.top_k)
    order = np.lexsort((cand_flat, -cand_vals))[:k]
    top_flat = cand_flat[order]
    top_vals = cand_vals[order]

    ref_corr_indices = (top_flat // M).astype(np.int32)
    src_corr_indices = (top_flat % M).astype(np.int32)
    corr_scores = np.exp(2.0 * top_vals - 2.0).astype(np.float32)

    return ref_corr_indices, src_corr_indices, corr_scores, all_ref_corr_indices


# revision 16
# speedup vs baseline: 1.4033x; 1.4033x over previous
"""CoarseMatching (retrieval kNN) kernel for 8x Trainium2 NeuronCores.

Problem: ref[8192,256], src[8192,256] (unit-norm rows, fp32).
  sim = ref @ src.T                      [8192, 8192]
  scores = exp(2*sim - 2)                (monotone in sim)
  outputs: global top-k (k=num_proposal) of scores (row idx, col idx, score)
           + per-row argmax over src.

Strategy:
  - Shard ref rows across 8 cores (1024 rows each); src replicated.
  - Device (per core): bf16 matmul (fp32 PSUM accumulation) of its
    [1024 x 8192] sim block. PSUM drain is split between the two engines
    that can read PSUM, each producing a per-row, per-512-column-chunk
    statistic:
      * VectorE: plain max-reduce -> exact chunk max (even chunks, plus
        every 3rd wave's odd chunk to balance engine load).
      * ScalarE: activation(Exp, scale=BETA, bias=-BETA*CC, accum_out)
        -> sum of exp(BETA*(x-CC)); the host turns this into a
        log-sum-exp upper estimate of the chunk max with bias in
        [0, ln(512)/BETA]. Overflow (inf) or underflow (0) simply makes
        the chunk an unconditional / guarded candidate.
  - Host: candidate selection from the chunk statistics with a margin
    that dominates all estimator errors, then exact fp64 recomputation
    of only the candidate chunks (BLAS) for exact top-k / argmax.

  Error budget: bf16 input rounding ~6e-4, exp-estimator bias
  ln(512)/BETA ~ 6.2e-3; MARGIN = 1.6e-2 dominates the sum.
"""

import sys

sys.path.insert(0, "/opt/trn_rl_repo")

import numpy as np
import ml_dtypes

N_CORES = 8
N, M, C = 8192, 8192, 256
ROWS_PER_CORE = N // N_CORES          # 1024
STRIPS = ROWS_PER_CORE // 128         # 8 strips of 128 rows
BANK = 512                            # fp32 elems per PSUM bank
N_PAIRS = M // (2 * BANK)             # 8 chunk-pairs
N_CHUNKS = M // BANK                  # 16 chunks of 512 columns
BETA = 1000.0                         # exp sharpness
CC = 0.25                             # exp center
MARGIN = 1.6e-2
UNDERFLOW_GUARD = CC - 87.0 / BETA + 0.01   # row-max below this -> exp may underflow
DVE_Q_CYCLE = 4                       # every 4th wave VectorE drains Q too

_compiled = None


def _wave_is_dve(t, u):
    return (t * (STRIPS // 2) + u) % DVE_Q_CYCLE == DVE_Q_CYCLE - 1


def _build_bass():
    from contextlib import ExitStack
    import concourse.bacc as bacc
    import concourse.tile as tile
    from concourse import mybir

    nc = bacc.Bacc("TRN2", target_bir_lowering=False, debug=False)
    bf16 = mybir.dt.bfloat16
    f32 = mybir.dt.float32
    MAX = mybir.AluOpType.max

    # lhsT k-tiles: [2, 128, 1024] (contract dim on partitions)
    ref_t = nc.declare_dram_parameter("ref_t", [2, 128, ROWS_PER_CORE], bf16, isOutput=False)
    # rhs k-tiles quartered for load/compute overlap: [2, 4, 128, 2048]
    src_t = nc.declare_dram_parameter("src_t", [2, 4, 128, M // 4], bf16, isOutput=False)
    # outputs: [128 partitions, pair t, 8 strips]
    cmv_out = nc.declare_dram_parameter("cmv", [128, N_PAIRS, STRIPS], f32, isOutput=True)
    cme_out = nc.declare_dram_parameter("cme", [128, N_PAIRS, STRIPS], f32, isOutput=True)

    with tile.TileContext(nc) as tc, ExitStack() as ctx:
        sbuf = ctx.enter_context(tc.tile_pool(name="sbuf", bufs=1))
        scr_pool = ctx.enter_context(tc.tile_pool(name="scr", bufs=2))
        psum_p = ctx.enter_context(tc.tile_pool(name="psp", bufs=2, space="PSUM"))
        psum_q = ctx.enter_context(tc.tile_pool(name="psq", bufs=2, space="PSUM"))

        bias_t = sbuf.tile([128, 1], f32, name="bias_t")
        nc.gpsimd.memset(bias_t[:], -BETA * CC)

        # resident weights (ref^T) per k-tile, then src^T quarters.
        # First-needed loads go out on distinct DMA-capable engines.
        reft = [sbuf.tile([128, ROWS_PER_CORE], bf16, name=f"reft{k}") for k in range(2)]
        srcq = [
            [sbuf.tile([128, M // 4], bf16, name=f"srcq{k}_{q}") for q in range(4)]
            for k in range(2)
        ]
        nc.scalar.dma_start(reft[0][:], ref_t[0])
        nc.gpsimd.dma_start(reft[1][:], ref_t[1])
        nc.sync.dma_start(srcq[0][0][:, :M // 8], src_t[0, 0][:, :M // 8])
        nc.scalar.dma_start(srcq[1][0][:, :M // 8], src_t[1, 0][:, :M // 8])
        nc.sync.dma_start(srcq[0][0][:, M // 8:], src_t[0, 0][:, M // 8:])
        nc.scalar.dma_start(srcq[1][0][:, M // 8:], src_t[1, 0][:, M // 8:])
        for q in range(1, 4):
            for k in range(2):
                nc.sync.dma_start(srcq[k][q][:], src_t[k, q])

        cmv_all = sbuf.tile([128, N_PAIRS, STRIPS], f32, name="cmv_all")
        cme_all = sbuf.tile([128, N_PAIRS, STRIPS], f32, name="cme_all")

        for t in range(N_PAIRS):              # column chunk-pairs (1024 cols)
            j0, j1 = 2 * t, 2 * t + 1
            q0, off0 = j0 // 4, (j0 % 4) * BANK
            q1, off1 = j1 // 4, (j1 % 4) * BANK
            for u in range(STRIPS // 2):      # strip pairs
                ps_p = psum_p.tile([128, 2, BANK], f32, name="ps_p", tag="ps_p")
                ps_q = psum_q.tile([128, 2, BANK], f32, name="ps_q", tag="ps_q")
                for s in range(2):
                    strip = 2 * u + s
                    rsl = slice(strip * 128, (strip + 1) * 128)
                    for k in range(2):
                        nc.tensor.matmul(
                            ps_p[:, s], reft[k][:, rsl],
                            srcq[k][q0][:, off0:off0 + BANK],
                            start=(k == 0), stop=(k == 1),
                        )
                for s in range(2):
                    strip = 2 * u + s
                    rsl = slice(strip * 128, (strip + 1) * 128)
                    for k in range(2):
                        nc.tensor.matmul(
                            ps_q[:, s], reft[k][:, rsl],
                            srcq[k][q1][:, off1:off1 + BANK],
                            start=(k == 0), stop=(k == 1),
                        )
                # VectorE: exact max of the even chunk (both strips at once)
                nc.vector.tensor_reduce(
                    cmv_all[:, t, 2 * u:2 * u + 2], ps_p[:, :, :],
                    axis=mybir.AxisListType.X, op=MAX,
                )
                if _wave_is_dve(t, u):
                    # VectorE also drains the odd chunk -> exact max
                    nc.vector.tensor_reduce(
                        cme_all[:, t, 2 * u:2 * u + 2], ps_q[:, :, :],
                        axis=mybir.AxisListType.X, op=MAX,
                    )
                else:
                    # ScalarE: exp-sum of the odd chunk, one bank per call
                    scr = scr_pool.tile([128, 2, BANK], bf16, name="scr", tag="scr")
                    for s in range(2):
                        nc.scalar.activation(
                            scr[:, s], ps_q[:, s],
                            mybir.ActivationFunctionType.Exp,
                            bias=bias_t[:], scale=BETA,
                            accum_out=cme_all[:, t, 2 * u + s:2 * u + s + 1],
                        )

        nc.sync.dma_start(cmv_out[:], cmv_all[:])
        nc.sync.dma_start(cme_out[:], cme_all[:])

    nc.compile()
    return nc


def _get_compiled():
    global _compiled
    if _compiled is None:
        _compiled = _build_bass()
    return _compiled


def _ensure_ntff_hook():
    """Register the axon NTFF profiling hook if the image's antenv lacks it."""
    try:
        from antenv.axon_hooks import get_axon_ntff_profile_hook  # noqa: F401
        return
    except ImportError:
        pass
    try:
        import types

        sys.path.insert(0, "/root/.axon_site")
        from trn_agent_boot.trn_boot import _ntff_profile_via_ctypes

        hook = _ntff_profile_via_ctypes("/opt/axon/libaxon_pjrt.so")
        m = types.ModuleType("antenv.axon_hooks")
        m._hook = hook
        m.get_axon_ntff_profile_hook = lambda: m._hook
        m.set_axon_ntff_profile_hook = lambda h: setattr(m, "_hook", h)
        sys.modules["antenv.axon_hooks"] = m
        import antenv

        antenv.axon_hooks = m
    except Exception as e:  # profiling is optional; never break the run
        print(f"NTFF hook registration failed: {e}", file=sys.stderr)


def _run_device(ref_f32: np.ndarray, src_f32: np.ndarray, trace: bool = False):
    """Run the SPMD bass kernel.

    Returns (cm [N, N_CHUNKS] fp64 chunk-max estimates, is_exp [N, N_CHUNKS]
    bool where the estimate is a log-sum-exp upper bound, results obj)."""
    from concourse.bass_utils import run_bass_kernel_spmd

    if trace:
        _ensure_ntff_hook()

    nc = _get_compiled()

    ref_bf = ref_f32.astype(ml_dtypes.bfloat16)
    src_bf = src_f32.astype(ml_dtypes.bfloat16)

    # [C, M] transposed layouts, k-tiled on partitions
    src_tt = np.ascontiguousarray(src_bf.T).reshape(2, 128, M)
    src_tt = np.ascontiguousarray(src_tt.reshape(2, 128, 4, M // 4).transpose(0, 2, 1, 3))

    in_maps = []
    for c in range(N_CORES):
        rows = slice(c * ROWS_PER_CORE, (c + 1) * ROWS_PER_CORE)
        reft = np.ascontiguousarray(ref_bf[rows].T).reshape(2, 128, ROWS_PER_CORE)
        in_maps.append({"ref_t": reft, "src_t": src_tt})

    res = run_bass_kernel_spmd(nc, in_maps, core_ids=list(range(N_CORES)), trace=trace)

    # block[t, p, i] -> local row = i*128 + p; chunk 2t (cmv) / 2t+1 (cme)
    cmv = np.empty((N, N_PAIRS), dtype=np.float32)
    cme = np.empty((N, N_PAIRS), dtype=np.float32)
    for c in range(N_CORES):
        rows = slice(c * ROWS_PER_CORE, (c + 1) * ROWS_PER_CORE)
        cmv[rows] = res.results[c]["cmv"].transpose(2, 0, 1).reshape(ROWS_PER_CORE, N_PAIRS)
        cme[rows] = res.results[c]["cme"].transpose(2, 0, 1).reshape(ROWS_PER_CORE, N_PAIRS)

    # odd-chunk statistic type by (t, u) wave: exact max vs exp estimate
    strip_of_row = (np.arange(N) % ROWS_PER_CORE) // 128
    u_of_row = strip_of_row // 2
    is_exp_pair = np.empty((N, N_PAIRS), dtype=bool)
    for t in range(N_PAIRS):
        is_exp_pair[:, t] = ~np.vectorize(lambda u: _wave_is_dve(t, u))(u_of_row)

    cm = np.empty((N, N_CHUNKS), dtype=np.float64)
    is_exp = np.zeros((N, N_CHUNKS), dtype=bool)
    cm[:, 0::2] = cmv
    odd = cme.astype(np.float64).copy()
    with np.errstate(divide="ignore"):
        est = np.log(odd) / BETA + CC
    est[~np.isfinite(odd)] = np.inf
    est[odd == 0.0] = -np.inf
    cm[:, 1::2] = np.where(is_exp_pair, est, odd)
    is_exp[:, 1::2] = is_exp_pair
    return cm, is_exp, res


def _recompute_chunks(ref64, src64, rows_arr, chunks_arr):
    """Exact fp64 sims for (row, chunk) pairs, grouped by chunk.

    Yields (chunk j, rows, values [len(rows), BANK])."""
    order = np.argsort(chunks_arr, kind="stable")
    rows_arr = rows_arr[order]
    chunks_arr = chunks_arr[order]
    bounds = np.searchsorted(chunks_arr, np.arange(N_CHUNKS + 1))
    for j in range(N_CHUNKS):
        lo, hi = bounds[j], bounds[j + 1]
        if lo == hi:
            continue
        rows = rows_arr[lo:hi]
        vals = ref64[rows] @ src64[j * BANK:(j + 1) * BANK].T
        yield j, rows, vals


def kernel(ref_feats, src_feats, num_proposal):
    ref = np.asarray(ref_feats, dtype=np.float32)
    src = np.asarray(src_feats, dtype=np.float32)
    k = int(num_proposal)

    cm, is_exp, _ = _run_device(ref, src)

    ref64 = ref.astype(np.float64)
    src64 = src.astype(np.float64)

    # ---- per-row argmax over src (all_ref_corr_indices) ----
    row_best = cm.max(axis=1)
    cand_mask = cm >= (row_best[:, None] - MARGIN)
    # underflow guard: if a row is weak enough that the exp path may have
    # underflowed, treat all its exp-estimated chunks as candidates
    risky = row_best < UNDERFLOW_GUARD
    if risky.any():
        cand_mask[risky] |= is_exp[risky]
    rows_r, chunks_r = np.nonzero(cand_mask)
    best_val = np.full(N, -np.inf)
    best_idx = np.zeros(N, dtype=np.int64)
    for j, rows, vals in _recompute_chunks(ref64, src64, rows_r, chunks_r):
        am = vals.argmax(axis=1)
        v = vals[np.arange(len(rows)), am]
        idx = j * BANK + am
        upd = v > best_val[rows]
        # strict > keeps the lowest column index on exact ties because
        # chunks are visited in ascending order and argmax takes the first max
        best_val[rows] = np.where(upd, v, best_val[rows])
        best_idx[rows] = np.where(upd, idx, best_idx[rows])
    all_ref_corr_indices = best_idx.astype(np.int32)

    # ---- global top-k ----
    flat_cm = cm.ravel()
    finite = np.isfinite(flat_cm)
    kth = min(k, int(finite.sum()))
    t_sel = np.partition(flat_cm[finite], finite.sum() - kth)[finite.sum() - kth]
    rows_g, chunks_g = np.nonzero(cm >= t_sel - MARGIN)
    cand_vals = []
    cand_flat = []
    for j, rows, vals in _recompute_chunks(ref64, src64, rows_g, chunks_g):
        cols = j * BANK + np.arange(BANK)
        cand_vals.append(vals.ravel())
        cand_flat.append((rows[:, None] * M + cols[None, :]).ravel())
    cand_vals = np.concatenate(cand_vals)
    cand_flat = np.concatenate(cand_flat)

    # top-k by value desc, ties -> lower flat index (matches jax.lax.top_k)
    order = np.lexsort((cand_flat, -cand_vals))[:k]
    top_flat = cand_flat[order]
    top_vals = cand_vals[order]

    ref_corr_indices = (top_flat // M).astype(np.int32)
    src_corr_indices = (top_flat % M).astype(np.int32)
    corr_scores = np.exp(2.0 * top_vals - 2.0).astype(np.float32)

    return ref_corr_indices, src_corr_indices, corr_scores, all_ref_corr_indices


# revision 19
# speedup vs baseline: 1.4145x; 1.0080x over previous
"""CoarseMatching (retrieval kNN) kernel for 8x Trainium2 NeuronCores.

Problem: ref[8192,256], src[8192,256] (unit-norm rows, fp32).
  sim = ref @ src.T                      [8192, 8192]
  scores = exp(2*sim - 2)                (monotone in sim)
  outputs: global top-k (k=num_proposal) of scores (row idx, col idx, score)
           + per-row argmax over src.

Strategy:
  - Shard ref rows across 8 cores (1024 rows each); src replicated.
  - Device (per core): bf16 matmul (fp32 PSUM accumulation) of its
    [1024 x 8192] sim block. PSUM drain is split between the two engines
    that can read PSUM, each producing a per-row, per-512-column-chunk
    statistic:
      * VectorE: plain max-reduce -> exact chunk max (even chunks, plus
        every 3rd wave's odd chunk to balance engine load).
      * ScalarE: activation(Exp, scale=BETA, bias=-BETA*CC, accum_out)
        -> sum of exp(BETA*(x-CC)); the host turns this into a
        log-sum-exp upper estimate of the chunk max with bias in
        [0, ln(512)/BETA]. Overflow (inf) or underflow (0) simply makes
        the chunk an unconditional / guarded candidate.
  - Host: candidate selection from the chunk statistics with a margin
    that dominates all estimator errors, then exact fp64 recomputation
    of only the candidate chunks (BLAS) for exact top-k / argmax.

  Error budget: bf16 input rounding ~6e-4, exp-estimator bias
  ln(512)/BETA ~ 6.2e-3; MARGIN = 1.6e-2 dominates the sum.
"""

import sys

sys.path.insert(0, "/opt/trn_rl_repo")

import numpy as np
import ml_dtypes

N_CORES = 8
N, M, C = 8192, 8192, 256
ROWS_PER_CORE = N // N_CORES          # 1024
STRIPS = ROWS_PER_CORE // 128         # 8 strips of 128 rows
BANK = 512                            # fp32 elems per PSUM bank
N_PAIRS = M // (2 * BANK)             # 8 chunk-pairs
N_CHUNKS = M // BANK                  # 16 chunks of 512 columns
BETA = 1000.0                         # exp sharpness
CC = 0.25                             # exp center
MARGIN = 1.6e-2
UNDERFLOW_GUARD = CC - 87.0 / BETA + 0.01   # row-max below this -> exp may underflow
DVE_Q_CYCLE = 3                       # every 3rd wave VectorE drains Q too

_compiled = None


def _wave_is_dve(t, u):
    return (t * (STRIPS // 2) + u) % DVE_Q_CYCLE == DVE_Q_CYCLE - 1


def _build_bass():
    from contextlib import ExitStack
    import concourse.bacc as bacc
    import concourse.tile as tile
    from concourse import mybir

    nc = bacc.Bacc("TRN2", target_bir_lowering=False, debug=False)
    bf16 = mybir.dt.bfloat16
    f32 = mybir.dt.float32
    MAX = mybir.AluOpType.max

    # lhsT k-tiles: [2, 128, 1024] (contract dim on partitions)
    ref_t = nc.declare_dram_parameter("ref_t", [2, 128, ROWS_PER_CORE], bf16, isOutput=False)
    # rhs k-tiles quartered for load/compute overlap: [2, 4, 128, 2048]
    src_t = nc.declare_dram_parameter("src_t", [2, 4, 128, M // 4], bf16, isOutput=False)
    # outputs: [128 partitions, pair t, 8 strips]
    cmv_out = nc.declare_dram_parameter("cmv", [128, N_PAIRS, STRIPS], f32, isOutput=True)
    cme_out = nc.declare_dram_parameter("cme", [128, N_PAIRS, STRIPS], f32, isOutput=True)

    with tile.TileContext(nc) as tc, ExitStack() as ctx:
        sbuf = ctx.enter_context(tc.tile_pool(name="sbuf", bufs=1))
        scr_pool = ctx.enter_context(tc.tile_pool(name="scr", bufs=2))
        psum_p = ctx.enter_context(tc.tile_pool(name="psp", bufs=2, space="PSUM"))
        psum_q = ctx.enter_context(tc.tile_pool(name="psq", bufs=2, space="PSUM"))

        bias_t = sbuf.tile([128, 1], f32, name="bias_t")
        nc.gpsimd.memset(bias_t[:], -BETA * CC)

        # resident weights (ref^T) per k-tile, then src^T quarters.
        # First-needed loads go out on distinct DMA-capable engines.
        reft = [sbuf.tile([128, ROWS_PER_CORE], bf16, name=f"reft{k}") for k in range(2)]
        srcq = [
            [sbuf.tile([128, M // 4], bf16, name=f"srcq{k}_{q}") for q in range(4)]
            for k in range(2)
        ]
        nc.scalar.dma_start(reft[0][:], ref_t[0])
        nc.gpsimd.dma_start(reft[1][:], ref_t[1])
        nc.sync.dma_start(srcq[0][0][:, :M // 8], src_t[0, 0][:, :M // 8])
        nc.scalar.dma_start(srcq[1][0][:, :M // 8], src_t[1, 0][:, :M // 8])
        nc.sync.dma_start(srcq[0][0][:, M // 8:], src_t[0, 0][:, M // 8:])
        nc.scalar.dma_start(srcq[1][0][:, M // 8:], src_t[1, 0][:, M // 8:])
        for q in range(1, 4):
            for k in range(2):
                nc.sync.dma_start(srcq[k][q][:], src_t[k, q])

        cmv_all = sbuf.tile([128, N_PAIRS, STRIPS], f32, name="cmv_all")
        cme_all = sbuf.tile([128, N_PAIRS, STRIPS], f32, name="cme_all")

        for t in range(N_PAIRS):              # column chunk-pairs (1024 cols)
            j0, j1 = 2 * t, 2 * t + 1
            q0, off0 = j0 // 4, (j0 % 4) * BANK
            q1, off1 = j1 // 4, (j1 % 4) * BANK
            for u in range(STRIPS // 2):      # strip pairs
                ps_p = psum_p.tile([128, 2, BANK], f32, name="ps_p", tag="ps_p")
                ps_q = psum_q.tile([128, 2, BANK], f32, name="ps_q", tag="ps_q")
                for s in range(2):
                    strip = 2 * u + s
                    rsl = slice(strip * 128, (strip + 1) * 128)
                    for k in range(2):
                        nc.tensor.matmul(
                            ps_p[:, s], reft[k][:, rsl],
                            srcq[k][q0][:, off0:off0 + BANK],
                            start=(k == 0), stop=(k == 1),
                        )
                for s in range(2):
                    strip = 2 * u + s
                    rsl = slice(strip * 128, (strip + 1) * 128)
                    for k in range(2):
                        nc.tensor.matmul(
                            ps_q[:, s], reft[k][:, rsl],
                            srcq[k][q1][:, off1:off1 + BANK],
                            start=(k == 0), stop=(k == 1),
                        )
                # VectorE: exact max of the even chunk (both strips at once)
                nc.vector.tensor_reduce(
                    cmv_all[:, t, 2 * u:2 * u + 2], ps_p[:, :, :],
                    axis=mybir.AxisListType.X, op=MAX,
                )
                if _wave_is_dve(t, u):
                    # VectorE also drains the odd chunk -> exact max
                    nc.vector.tensor_reduce(
                        cme_all[:, t, 2 * u:2 * u + 2], ps_q[:, :, :],
                        axis=mybir.AxisListType.X, op=MAX,
                    )
                else:
                    # ScalarE: exp-sum of the odd chunk, one bank per call
                    scr = scr_pool.tile([128, 2, BANK], bf16, name="scr", tag="scr")
                    for s in range(2):
                        nc.scalar.activation(
                            scr[:, s], ps_q[:, s],
                            mybir.ActivationFunctionType.Exp,
                            bias=bias_t[:], scale=BETA,
                            accum_out=cme_all[:, t, 2 * u + s:2 * u + s + 1],
                        )

        nc.sync.dma_start(cmv_out[:], cmv_all[:])
        nc.sync.dma_start(cme_out[:], cme_all[:])

    nc.compile()
    return nc


def _get_compiled():
    global _compiled
    if _compiled is None:
        _compiled = _build_bass()
    return _compiled


def _ensure_ntff_hook():
    """Register the axon NTFF profiling hook if the image's antenv lacks it."""
    try:
        from antenv.axon_hooks import get_axon_ntff_profile_hook  # noqa: F401
        return
    except ImportError:
        pass
    try:
        import types

        sys.path.insert(0, "/root/.axon_site")
        from trn_agent_boot.trn_boot import _ntff_profile_via_ctypes

        hook = _ntff_profile_via_ctypes("/opt/axon/libaxon_pjrt.so")
        m = types.ModuleType("antenv.axon_hooks")
        m._hook = hook
        m.get_axon_ntff_profile_hook = lambda: m._hook
        m.set_axon_ntff_profile_hook = lambda h: setattr(m, "_hook", h)
        sys.modules["antenv.axon_hooks"] = m
        import antenv

        antenv.axon_hooks = m
    except Exception as e:  # profiling is optional; never break the run
        print(f"NTFF hook registration failed: {e}", file=sys.stderr)


def _run_device(ref_f32: np.ndarray, src_f32: np.ndarray, trace: bool = False):
    """Run the SPMD bass kernel.

    Returns (cm [N, N_CHUNKS] fp64 chunk-max estimates, is_exp [N, N_CHUNKS]
    bool where the estimate is a log-sum-exp upper bound, results obj)."""
    from concourse.bass_utils import run_bass_kernel_spmd

    if trace:
        _ensure_ntff_hook()

    nc = _get_compiled()

    ref_bf = ref_f32.astype(ml_dtypes.bfloat16)
    src_bf = src_f32.astype(ml_dtypes.bfloat16)

    # [C, M] transposed layouts, k-tiled on partitions
    src_tt = np.ascontiguousarray(src_bf.T).reshape(2, 128, M)
    src_tt = np.ascontiguousarray(src_tt.reshape(2, 128, 4, M // 4).transpose(0, 2, 1, 3))

    in_maps = []
    for c in range(N_CORES):
        rows = slice(c * ROWS_PER_CORE, (c + 1) * ROWS_PER_CORE)
        reft = np.ascontiguousarray(ref_bf[rows].T).reshape(2, 128, ROWS_PER_CORE)
        in_maps.append({"ref_t": reft, "src_t": src_tt})

    res = run_bass_kernel_spmd(nc, in_maps, core_ids=list(range(N_CORES)), trace=trace)

    # block[t, p, i] -> local row = i*128 + p; chunk 2t (cmv) / 2t+1 (cme)
    cmv = np.empty((N, N_PAIRS), dtype=np.float32)
    cme = np.empty((N, N_PAIRS), dtype=np.float32)
    for c in range(N_CORES):
        rows = slice(c * ROWS_PER_CORE, (c + 1) * ROWS_PER_CORE)
        cmv[rows] = res.results[c]["cmv"].transpose(2, 0, 1).reshape(ROWS_PER_CORE, N_PAIRS)
        cme[rows] = res.results[c]["cme"].transpose(2, 0, 1).reshape(ROWS_PER_CORE, N_PAIRS)

    # odd-chunk statistic type by (t, u) wave: exact max vs exp estimate
    strip_of_row = (np.arange(N) % ROWS_PER_CORE) // 128
    u_of_row = strip_of_row // 2
    is_exp_pair = np.empty((N, N_PAIRS), dtype=bool)
    for t in range(N_PAIRS):
        is_exp_pair[:, t] = ~np.vectorize(lambda u: _wave_is_dve(t, u))(u_of_row)

    cm = np.empty((N, N_CHUNKS), dtype=np.float64)
    is_exp = np.zeros((N, N_CHUNKS), dtype=bool)
    cm[:, 0::2] = cmv
    odd = cme.astype(np.float64).copy()
    with np.errstate(divide="ignore"):
        est = np.log(odd) / BETA + CC
    est[~np.isfinite(odd)] = np.inf
    est[odd == 0.0] = -np.inf
    cm[:, 1::2] = np.where(is_exp_pair, est, odd)
    is_exp[:, 1::2] = is_exp_pair
    return cm, is_exp, res


def _recompute_chunks(ref64, src64, rows_arr, chunks_arr):
    """Exact fp64 sims for (row, chunk) pairs, grouped by chunk.

    Yields (chunk j, rows, values [len(rows), BANK])."""
    order = np.argsort(chunks_arr, kind="stable")
    rows_arr = rows_arr[order]
    chunks_arr = chunks_arr[order]
    bounds = np.searchsorted(chunks_arr, np.arange(N_CHUNKS + 1))
    for j in range(N_CHUNKS):
        lo, hi = bounds[j], bounds[j + 1]
        if lo == hi:
            continue
        rows = rows_arr[lo:hi]
        vals = ref64[rows] @ src64[j * BANK:(j + 1) * BANK].T
        yield j, rows, vals


def kernel(ref_feats, src_feats, num_proposal):
    ref = np.asarray(ref_feats, dtype=np.float32)
    src = np.asarray(src_feats, dtype=np.float32)
    k = int(num_proposal)

    cm, is_exp, _ = _run_device(ref, src)

    ref64 = ref.astype(np.float64)
    src64 = src.astype(np.float64)

    # ---- per-row argmax over src (all_ref_corr_indices) ----
    row_best = cm.max(axis=1)
    cand_mask = cm >= (row_best[:, None] - MARGIN)
    # underflow guard: if a row is weak enough that the exp path may have
    # underflowed, treat all its exp-estimated chunks as candidates
    risky = row_best < UNDERFLOW_GUARD
    if risky.any():
        cand_mask[risky] |= is_exp[risky]
    rows_r, chunks_r = np.nonzero(cand_mask)
    best_val = np.full(N, -np.inf)
    best_idx = np.zeros(N, dtype=np.int64)
    for j, rows, vals in _recompute_chunks(ref64, src64, rows_r, chunks_r):
        am = vals.argmax(axis=1)
        v = vals[np.arange(len(rows)), am]
        idx = j * BANK + am
        upd = v > best_val[rows]
        # strict > keeps the lowest column index on exact ties because
        # chunks are visited in ascending order and argmax takes the first max
        best_val[rows] = np.where(upd, v, best_val[rows])
        best_idx[rows] = np.where(upd, idx, best_idx[rows])
    all_ref_corr_indices = best_idx.astype(np.int32)

    # ---- global top-k ----
    flat_cm = cm.ravel()
    finite = np.isfinite(flat_cm)
    kth = min(k, int(finite.sum()))
    t_sel = np.partition(flat_cm[finite], finite.sum() - kth)[finite.sum() - kth]
    rows_g, chunks_g = np.nonzero(cm >= t_sel - MARGIN)
    cand_vals = []
    cand_flat = []
    for j, rows, vals in _recompute_chunks(ref64, src64, rows_g, chunks_g):
        cols = j * BANK + np.arange(BANK)
        cand_vals.append(vals.ravel())
        cand_flat.append((rows[:, None] * M + cols[None, :]).ravel())
    cand_vals = np.concatenate(cand_vals)
    cand_flat = np.concatenate(cand_flat)

    # top-k by value desc, ties -> lower flat index (matches jax.lax.top_k)
    order = np.lexsort((cand_flat, -cand_vals))[:k]
    top_flat = cand_flat[order]
    top_vals = cand_vals[order]

    ref_corr_indices = (top_flat // M).astype(np.int32)
    src_corr_indices = (top_flat % M).astype(np.int32)
    corr_scores = np.exp(2.0 * top_vals - 2.0).astype(np.float32)

    return ref_corr_indices, src_corr_indices, corr_scores, all_ref_corr_indices
